# revision 28
# baseline (speedup 1.0000x reference)
"""EnhancedMultiHeadAttention TRN2 kernel (8 NeuronCores), v2.

Problem (hardcoded): B=4, L=1024, HID=1024, H=16, DH=64, MAX_SEQ=1024.
  q/k/v = x @ W* + b*          (per-head split)
  S = q k^T / sqrt(64) + einsum('bhid,ijd->bhij', q, rel_emb[i-j+1023])
  attn = softmax(S); out = (attn @ v) @ Wo + bo

Sharding: core c -> batch b = c//2, head group g = c%2 (8 heads each).
Each core computes a partial output x[b]-block @ Wo-rows; host sums the
two partials per batch and adds bo.

v2 changes vs baseline (345us):
 - all HBM inputs bf16 (x, W's, rel table); biases f32.  Halves input DMA.
 - S and qE matmuls (contraction=64) issued as adjacent h0/h1 pairs with
   explicit tile_position (0,0)/(64,0): 2x row-tiled PE concurrency.
 - rel-bias trick as baseline: qE = q @ flip(rel)^T, then a skew SBUF->SBUF
   DMA re-read (row stride 1151 in a [128,1152] window) builds bias tiles.
 - per-(t,h) attn transposes (256KB each) spread through the score phase
   instead of per-head 2MB monoliths (those serialized ~20us/pair).
 - no per-attn-tile renorm: AV consumes unnormalized exp(s); ctx is scaled
   once per pair by 1/rowsum broadcast via two small DMAs.
 - PSUM: one pool, tag "mm" bufs=4 x [128,512] + tag "s2" bufs=2 x
   [128,1024] (2 banks) so the score bias-add is a single DVE op.
 - PSUM evacuation pinned/rotated across Scalar/Vector/GpSimd.
 - output projection written PSUM->DRAM directly; bo added on host.
"""

import ml_dtypes
import numpy as np

import concourse.bass as bass
import concourse.mybir as mybir
import concourse.tile as tile
from concourse.bass_utils import run_bass_kernel_spmd

B, L, HID, H = 4, 1024, 1024, 16
DH = 64
NPAIR = 4        # head pairs per core (8 heads = 4 pairs of 2)
NT = L // 128    # 8 i-tiles
F32 = mybir.dt.float32
BF16 = mybir.dt.bfloat16

ROW_TILE = True       # explicit tile_position row tiling for K=64 matmuls
DIRECT_OUT = False    # DMA PSUM -> DRAM unsupported (bass asserts SBUF/DRAM)

_uid = [0]


def _split_multi_waits(nc):
    """Installed walrus accepts 1 sync-wait per instruction (2 for
    EventSemaphore); Tile's tail drain can carry more. Spill extras onto
    EventSemaphore wait-carriers inserted before the offender."""
    for f in nc.m.functions:
        for blk in f.blocks:
            insts = blk.instructions
            idx = 0
            while idx < len(insts):
                inst = insts[idx]
                si = inst.sync_info
                waits = list(si.on_wait) if si and si.on_wait else []
                cap = 2 if type(inst).__name__ == "InstEventSemaphore" else 1
                if len(waits) > cap:
                    si.on_wait = waits[:cap]
                    extra = waits[cap:]
                    carriers = []
                    for k in range(0, len(extra), 2):
                        _uid[0] += 1
                        nop = mybir.InstEventSemaphore(
                            name=f"wait_split_{_uid[0]}", ins=[], outs=[]
                        )
                        nop.engine = inst.engine
                        nop.sync_info = mybir.SyncInfo(
                            on_wait=extra[k:k + 2], on_update=[]
                        )
                        carriers.append(nop)
                    for c in reversed(carriers):
                        insts.insert(idx, c)
                    idx += len(carriers)
                idx += 1


def _ap_with(ap, dims, offset=None):
    """Return a copy of `ap` with raw [step,count] dims and element offset
    (offset=None keeps the AP's own offset)."""
    c = ap.copy()
    v = c.ap
    while len(v) > len(dims):
        v.pop()
    n = len(v)
    for i in range(n):
        v[i] = list(dims[i])
    for d in list(dims)[n:]:
        v.append(list(d))
    c.ap = v
    if offset is not None:
        c.offset = offset
    return c


def _build_program():
    nc = bass.Bass()

    xT = nc.dram_tensor("xT", (HID, L), BF16, kind="ExternalInput")
    wq = nc.dram_tensor("wq", (HID, 512), BF16, kind="ExternalInput")
    wk = nc.dram_tensor("wk", (HID, 512), BF16, kind="ExternalInput")
    wv = nc.dram_tensor("wv", (HID, 512), BF16, kind="ExternalInput")
    wo = nc.dram_tensor("wo", (512, L), BF16, kind="ExternalInput")
    rt = nc.dram_tensor("rt", (128, 2048), BF16, kind="ExternalInput")
    bq = nc.dram_tensor("bq", (512,), F32, kind="ExternalInput")
    bk = nc.dram_tensor("bk", (512,), F32, kind="ExternalInput")
    bv = nc.dram_tensor("bv", (512,), F32, kind="ExternalInput")
    out = nc.dram_tensor("out", (L, L), F32, kind="ExternalOutput")

    def tp(h):
        return (64 * h, 0) if ROW_TILE else None

    with tile.TileContext(nc) as tc:
        with tc.tile_pool(name="weights", bufs=1) as wpool, \
             tc.tile_pool(name="proj", bufs=1) as projpool, \
             tc.tile_pool(name="ps", bufs=1, space="PSUM") as ps:

            # ---- warmup on a memset tile: no HBM dependency ----
            wu_sb = wpool.tile([128, 512], BF16)
            nc.gpsimd.memset(wu_sb[:], 0.0)
            wu_ps = ps.tile([128, 512], F32, tag="mm", bufs=3, name="wu_ps")
            for wi in range(16):
                nc.tensor.matmul(wu_ps[:], wu_sb[:, 0:128], wu_sb[:],
                                 start=(wi == 0), stop=(wi == 15))

            # ---- projections: QT/KT [qdim part, seq], V [seq part, vdim] ----
            qt_sb = [projpool.tile([128, L], BF16, name=f"qt{m}") for m in range(4)]
            kt_sb = [projpool.tile([128, L], BF16, name=f"kt{m}") for m in range(4)]
            v_sb = [projpool.tile([128, 512], BF16, name=f"v{t}")
                    for t in range(NT)]

            with tc.tile_pool(name="p1", bufs=1) as p1:
                xk = [p1.tile([128, L], BF16, name=f"xk{k}") for k in range(8)]
                wqk = [p1.tile([128, 512], BF16, name=f"wqk{k}") for k in range(8)]
                wkk = [p1.tile([128, 512], BF16, name=f"wkk{k}") for k in range(8)]
                wvk = [p1.tile([128, 512], BF16, name=f"wvk{k}") for k in range(8)]
                for k in range(8):
                    ksl = slice(k * 128, (k + 1) * 128)
                    nc.sync.dma_start(xk[k][:], xT[ksl, :])
                    nc.sync.dma_start(wqk[k][:], wq[ksl, :])
                    nc.sync.dma_start(wkk[k][:], wk[ksl, :])
                    nc.sync.dma_start(wvk[k][:], wv[ksl, :])

                # resident small tensors (after the first x/w slices so the
                # projection start isn't delayed behind the rel table)
                rt_sb = wpool.tile([128, 2048], BF16)
                nc.sync.dma_start(rt_sb[:], rt[:])
                bq_sb = wpool.tile([128, 4], F32)
                nc.sync.dma_start(bq_sb[:],
                                  bq[:].rearrange("(t p) -> p t", p=128))
                bk_sb = wpool.tile([128, 4], F32)
                nc.sync.dma_start(bk_sb[:],
                                  bk[:].rearrange("(t p) -> p t", p=128))
                # bv replicated across partitions: [1,512], partition step 0
                bv_sb = wpool.tile([128, 512], F32)
                nc.sync.dma_start(bv_sb[:],
                                  _ap_with(bv[None, :], [[0, 128], [1, 512]], 0))

                for m in range(4):
                    msl = slice(m * 128, (m + 1) * 128)
                    for c in range(2):
                        csl = slice(c * 512, (c + 1) * 512)
                        qt_ps = ps.tile([128, 512], F32, tag="mm",
                                        bufs=3, name=f"qtp{m}{c}")
                        kt_ps = ps.tile([128, 512], F32, tag="mm",
                                        bufs=3, name=f"ktp{m}{c}")
                        for k in range(8):
                            nc.tensor.matmul(qt_ps[:], wqk[k][:, msl],
                                             xk[k][:, csl],
                                             start=(k == 0), stop=(k == 7))
                        for k in range(8):
                            nc.tensor.matmul(kt_ps[:], wkk[k][:, msl],
                                             xk[k][:, csl],
                                             start=(k == 0), stop=(k == 7))
                        nc.vector.tensor_scalar_add(qt_sb[m][:, csl], qt_ps[:],
                                                    bq_sb[:, m:m + 1])
                        nc.vector.tensor_scalar_add(kt_sb[m][:, csl], kt_ps[:],
                                                    bk_sb[:, m:m + 1])
                for t in range(NT):
                    tsl = slice(t * 128, (t + 1) * 128)
                    v_ps = ps.tile([128, 512], F32, tag="mm", bufs=3,
                                   name=f"vp{t}")
                    for k in range(8):
                        nc.tensor.matmul(v_ps[:], xk[k][:, tsl], wvk[k][:],
                                         start=(k == 0), stop=(k == 7))
                    nc.vector.tensor_tensor(
                        v_sb[t][:], v_ps[:], bv_sb[:], mybir.AluOpType.add)

            # Wo loads trickle in during attention
            wo_sb = [wpool.tile([128, L], BF16, name=f"wo{m}") for m in range(4)]
            for m in range(4):
                nc.sync.dma_start(wo_sb[m][:], wo[m * 128:(m + 1) * 128, :])

            # ---- attention per head pair ----
            work = tc.alloc_tile_pool(name="work", bufs=3)
            apool = tc.alloc_tile_pool(name="attn", bufs=2)
            ctxT_sb = [None] * NPAIR

            # evac engine rotation for qE PSUM->SBUF chunks.
            # GPSIMD cannot access PSUM, so only Scalar (1.2GHz) and DVE
            # (0.96GHz) share evacuation; Scalar gets the larger share since
            # DVE also owns the score bias-adds.
            evac_engines = [nc.scalar, nc.scalar, nc.scalar, nc.vector]
            evac_idx = [0]

            def evac(dst, src):
                eng = evac_engines[evac_idx[0] % len(evac_engines)]
                evac_idx[0] += 1
                if eng is nc.scalar:
                    nc.scalar.copy(dst, src)
                else:
                    eng.tensor_copy(dst, src)

            QE_CH = ((0, 512), (512, 384), (896, 256))

            def emit_qe(p, t):
                """qE for both heads of pair p, i-tile t, row-tiled pairs.
                Returns the two bias tiles (skew-gathered)."""
                qt_p = qt_sb[p]
                w0 = 896 - 128 * t
                qe_sbs = []
                qe_pss = {}
                for ci, (c0, cw) in enumerate(QE_CH):
                    for h in range(2):
                        hs = slice(64 * h, 64 * h + 64)
                        isl = slice(t * 128, (t + 1) * 128)
                        qe_ps = ps.tile([128, 512], F32, tag="mm",
                                        bufs=3, name=f"qe{p}{t}{h}{ci}")
                        nc.tensor.matmul(
                            qe_ps[:, :cw], qt_p[hs, isl],
                            rt_sb[hs, w0 + c0:w0 + c0 + cw],
                            start=True, stop=True, tile_position=tp(h))
                        qe_pss[(ci, h)] = qe_ps
                for h in range(2):
                    qe_sb = work.tile([128, 1152], BF16, tag="qe", bufs=4,
                                      name=f"qesb{h}")
                    qe_sbs.append(qe_sb)
                    for ci, (c0, cw) in enumerate(QE_CH):
                        evac(qe_sb[:, c0:c0 + cw], qe_pss[(ci, h)][:, :cw])
                # skew gather: bias[q, j] = qe_sb[q, 127 - q + j]
                biases = []
                for h in range(2):
                    bias_sb = work.tile([128, L], BF16, tag="bias", bufs=7,
                                        name=f"bias{h}")
                    nc.sync.dma_start(
                        bias_sb[:],
                        _ap_with(qe_sbs[h][:, 0:1024],
                                 [[1151, 128], [1, 1024]], 127))
                    biases.append(bias_sb)
                return biases

            def emit_s(p, t, biases, attn_pool, aT, sums_h):
                """Scores+exp+renorm for both heads of pair p, i-tile t."""
                qt_p = qt_sb[p]
                kt_p = kt_sb[p]
                isl = slice(t * 128, (t + 1) * 128)
                s2 = []
                for h in range(2):
                    s2.append(ps.tile([128, 1024], F32, tag="s2", bufs=2,
                                      name=f"s2_{h}"))
                for c in range(2):
                    csl = slice(c * 512, (c + 1) * 512)
                    for h in range(2):
                        hs = slice(64 * h, 64 * h + 64)
                        nc.tensor.matmul(
                            s2[h][:, csl], qt_p[hs, isl], kt_p[hs, csl],
                            start=True, stop=True, tile_position=tp(h))
                for h in range(2):
                    s_sb = work.tile([128, L], F32, tag="ssb", bufs=3,
                                     name="s_sb")
                    nc.vector.tensor_tensor(
                        s_sb[:], s2[h][:], biases[h][:], mybir.AluOpType.add)
                    attn_t = attn_pool.tile([128, L], BF16, tag="attn",
                                            bufs=5, name=f"attn{h}")
                    nc.scalar.activation(
                        attn_t[:], s_sb[:],
                        mybir.ActivationFunctionType.Exp,
                        accum_out=sums_h[h][:, t:t + 1])
                    recip = work.tile([128, 1], F32, tag="recip", bufs=4,
                                      name="recip")
                    nc.vector.reciprocal(recip[:], sums_h[h][:, t:t + 1])
                    # bf16 all-SBUF renorm multiply hits the 4x DVE mode
                    nc.vector.tensor_scalar_mul(attn_t[:], attn_t[:],
                                                recip[:])
                    # per-tile block transpose into aT[h][:, t]
                    nc.sync.dma_start(aT[h][:, t], attn_t[:], transpose=True)

            def av_chunk(p, aT, ctx, c):
                """AV for output i-chunk c of pair p (needs transposes of
                t in [4c, 4c+4) only)."""
                csl = slice(c * 512, (c + 1) * 512)
                ctx_ps = ps.tile([128, 512], F32, tag="ctx", bufs=1,
                                 name=f"ctx{p}{c}")
                for h in range(2):
                    hh = 2 * p + h
                    for jt in range(NT):
                        nc.tensor.matmul(
                            ctx_ps[64 * h:64 * h + 64, :],
                            v_sb[jt][:, 64 * hh:64 * hh + 64],
                            aT[h][:, 4 * c:4 * (c + 1), jt, :],
                            start=(jt == 0), stop=(jt == NT - 1))
                nc.vector.tensor_copy(ctx[:, csl], ctx_ps[:])

            # Flat software pipeline over (pair, t): qE runs QPF iterations
            # ahead of the consuming score block, crossing pair boundaries.
            # AV chunks for pair p are emitted DAV iterations into pair p+1's
            # score phase so the PE never waits on pair p's exp/renorm/
            # transpose tail.
            QPF = 3
            items = [(p, t) for p in range(NPAIR) for t in range(NT)]
            aT_by_p = {}
            sums_by_p = {}
            ctx_by_p = {}
            bias_tiles = {}

            def consume(idx):
                p, t = items[idx]
                emit_s(p, t, bias_tiles.pop(idx), apool,
                       aT_by_p[p], sums_by_p[p])
                if p > 0:
                    if t == 1:
                        av_chunk(p - 1, aT_by_p[p - 1], ctx_by_p[p - 1], 0)
                    elif t == 3:
                        av_chunk(p - 1, aT_by_p.pop(p - 1),
                                 ctx_by_p[p - 1], 1)

            for idx in range(len(items) + QPF):
                if idx < len(items):
                    p, t = items[idx]
                    if t == 0:
                        # aT[h] layout: [j-local 128, t, jt, i-local 128]
                        aT_by_p[p] = [
                            apool.tile([128, NT, NT, 128], BF16, tag="aT",
                                       bufs=2, name=f"aT{p}_{h}")
                            for h in range(2)]
                        sums_by_p[p] = [
                            work.tile([128, NT], F32, tag="sums", bufs=4,
                                      name=f"sums{p}_{h}")
                            for h in range(2)]
                        ctx_by_p[p] = projpool.tile([128, L], BF16,
                                                    name=f"ctxT{p}")
                        ctxT_sb[p] = ctx_by_p[p]
                    bias_tiles[idx] = emit_qe(p, t)
                if idx >= QPF:
                    consume(idx - QPF)
            # tail: AV for the last pair
            av_chunk(NPAIR - 1, aT_by_p[NPAIR - 1], ctx_by_p[NPAIR - 1], 0)
            av_chunk(NPAIR - 1, aT_by_p.pop(NPAIR - 1),
                     ctx_by_p[NPAIR - 1], 1)

            # ---- output projection: ctxT[hd,i] x Wo[hd,o], direct to DRAM ----
            opool = tc.alloc_tile_pool(name="outp", bufs=3)
            for t in range(NT):
                isl = slice(t * 128, (t + 1) * 128)
                for c in range(2):
                    o_ps = ps.tile([128, 512], F32, tag="mm", bufs=3,
                                   name=f"o{t}{c}")
                    for m in range(4):
                        nc.tensor.matmul(
                            o_ps[:], ctxT_sb[m][:, isl],
                            wo_sb[m][:, c * 512:(c + 1) * 512],
                            start=(m == 0), stop=(m == 3))
                    if DIRECT_OUT:
                        nc.sync.dma_start(out[isl, c * 512:(c + 1) * 512],
                                          o_ps[:])
                    else:
                        o_sb = opool.tile([128, 512], F32, tag="osb",
                                          bufs=4)
                        nc.vector.tensor_copy(o_sb[:], o_ps[:])
                        nc.sync.dma_start(out[isl, c * 512:(c + 1) * 512],
                                          o_sb[:])
            opool.release()
            apool.release()
            work.release()

    _split_multi_waits(nc)
    return nc


_cached = {}


def _get_program():
    if "nc" not in _cached:
        _cached["nc"] = _build_program()
    return _cached["nc"]


def kernel(x, Wq, bq, Wk, bk, Wv, bv, Wo, bo, rel_emb, _timing=None):
    x = np.asarray(x, np.float32)
    Wq = np.asarray(Wq, np.float32)
    Wk = np.asarray(Wk, np.float32)
    Wv = np.asarray(Wv, np.float32)
    Wo = np.asarray(Wo, np.float32)
    bq_ = np.asarray(bq, np.float32)
    bk_ = np.asarray(bk, np.float32)
    bv_ = np.asarray(bv, np.float32)
    bo_ = np.asarray(bo, np.float32)
    rel = np.asarray(rel_emb, np.float32)

    # flipped rel table, transposed, duplicated on both 64-partition halves,
    # padded to 2048 cols
    rt_half = rel[::-1, :].T  # [64, 2047]
    rt_np = np.zeros((128, 2048), ml_dtypes.bfloat16)
    rt_np[0:64, 0:2047] = rt_half.astype(ml_dtypes.bfloat16)
    rt_np[64:128, 0:2047] = rt_half.astype(ml_dtypes.bfloat16)

    bf = ml_dtypes.bfloat16
    in_maps = []
    for core in range(8):
        b, g = divmod(core, 2)
        cols = slice(g * 512, (g + 1) * 512)
        in_maps.append({
            "xT": np.ascontiguousarray(x[b].T).astype(bf),
            "wq": np.ascontiguousarray(Wq[:, cols]).astype(bf),
            "wk": (np.ascontiguousarray(Wk[:, cols]) / 8.0).astype(bf),
            "wv": np.ascontiguousarray(Wv[:, cols]).astype(bf),
            "wo": np.ascontiguousarray(Wo[cols, :]).astype(bf),
            "rt": rt_np,
            "bq": np.ascontiguousarray(bq_[cols]),
            "bk": np.ascontiguousarray(bk_[cols]) / 8.0,
            "bv": np.ascontiguousarray(bv_[cols]),
        })

    nc = _get_program()
    kwargs = {}
    if _timing is not None:
        kwargs = dict(trace=True, trace_cores=list(range(8)))
    r = run_bass_kernel_spmd(nc, in_maps, core_ids=list(range(8)), **kwargs)
    if _timing is not None:
        _timing["exec_time_ns"] = r.exec_time_ns
        _timing["mean_exec_time_ns"] = r.mean_exec_time_ns
        _timing["trace"] = r.instructions_and_trace
    outs = [r.results[c]["out"] for c in range(8)]
    return np.stack(
        [outs[2 * b] + outs[2 * b + 1] + bo_[None, :] for b in range(B)],
        axis=0)
